# revision 13
# baseline (speedup 1.0000x reference)
"""RNN-T Joint network kernel for Trainium2 (Bass/Tile), 8-core SPMD.

Problem: out[b,t,u,v] = tanh(enc[b,t,:] + pred[b,u,:]) @ W[v,:] + bias[v]
  B=4, T=256, U=64, D=640, V=4096  (fp32)

Sharding: data-parallel over (B,T). Core i handles b = i//2, t in
[(i%2)*128, (i%2)*128+128). Each core computes an [128*64, 4096] slice of
the output; no collectives needed.

Device kernel (per core):
  - host pre-transposes operands so the contraction dim D sits on SBUF
    partitions: encT [D,128], predT [D,64], wT [D,V].
  - hT[d, (t,u)] = tanh(predT[d,u] + encT[d,t]) via scalar-engine
    activation with per-partition bias (one instr per (d-chunk, t)).
  - PE matmul: psum[m128, n512] += hT[k][:,m].T @ wT[k][:,n] over 5
    k-chunks of 128; 8 psum banks cover V=4096 per 128-row chunk.
  - drain: vector add bias (broadcast tile) PSUM->SBUF, DMA out.
"""

import os
import sys

import numpy as np

if "/root/.axon_site/_ro/trn_rl_repo" not in sys.path:
    sys.path.append("/root/.axon_site/_ro/trn_rl_repo")

import concourse.bass as bass  # noqa: E402
import concourse.mybir as mybir  # noqa: E402
import concourse.tile as tile  # noqa: E402
from concourse import bacc  # noqa: E402
from concourse.bass_utils import run_bass_kernel_spmd  # noqa: E402

B, T, U, D, V = 4, 256, 64, 640, 4096
N_CORES = 8
T_PER_CORE = T // (N_CORES // B)  # 128
ROWS = T_PER_CORE * U  # 8192 rows per core
KC = D // 128  # 5 k-chunks
NB = V // 512  # 8 psum banks per row-chunk
M_CHUNKS = ROWS // 128  # 64  (each = 2 t values x 64 u)
T_PER_M = 128 // U  # 2

# matmul dtype mode: "f32" (exact, 4 cyc/row), "f32r" (1 cyc/row @N>=256),
# "bf16" (1 cyc/row, operands rounded to bf16, half the weight DMA)
MM_MODE = os.environ.get("JOINT_MM_MODE", "bf16")


def build_nc(mode: str, dyn_iters: bool = False):
    """dyn_iters=True adds a runtime `niters` input and wraps the whole
    per-core compute in a device-side For_i — used only for wall-clock
    timing (per-iter time = delta between two niters values)."""
    nc = bacc.Bacc("TRN2", target_bir_lowering=False, debug=False)

    f32 = mybir.dt.float32
    bf16 = mybir.dt.bfloat16
    f32r = mybir.dt.float32r
    w_dt = {"bf16": bf16, "f32r": f32r}.get(mode, f32)
    h_dt = w_dt

    b_dt = bf16 if mode == "bf16" else f32
    encT_d = nc.dram_tensor("encT", [D, T_PER_CORE], f32, kind="ExternalInput")
    predT_d = nc.dram_tensor("predT", [D, U], f32, kind="ExternalInput")
    wT_d = nc.dram_tensor("wT", [D, V], w_dt, kind="ExternalInput")
    bias_d = nc.dram_tensor("bias", [1, V], b_dt, kind="ExternalInput")
    if dyn_iters:
        n_d = nc.dram_tensor("niters", [1, 1], mybir.dt.int32, kind="ExternalInput")
        # timing build: full-size DRAM writes still happen (same DMA traffic),
        # but the 128MB buffer stays device-internal; only a small probe ships
        # back so wall-clock isn't drowned by tunnel transfer.
        out_d = nc.dram_tensor("out", [ROWS, V], f32, kind="Internal")
        probe_d = nc.dram_tensor("probe", [128, 512], b_dt, kind="ExternalOutput")
    else:
        out_d = nc.dram_tensor("out", [ROWS, V], f32, kind="ExternalOutput")

    encT = encT_d.ap().rearrange("(k p) t -> p k t", p=128)
    predT = predT_d.ap().rearrange("(k p) u -> p k u", p=128)
    wT = wT_d.ap().rearrange("(k p) v -> p k v", p=128)
    out = out_d.ap()

    with tile.TileContext(nc) as tc:
        with (
            tc.tile_pool(name="singles", bufs=1) as singles,
            tc.tile_pool(name="hpool", bufs=3) as hpool,
            tc.tile_pool(name="opool", bufs=3) as opool,
            tc.tile_pool(name="lastp", bufs=1) as lastp,
            tc.tile_pool(name="psum", bufs=8, space="PSUM") as psum_pool,
        ):
            # small operands first so the activation pipe starts immediately;
            # weights follow per-k so early k-sweeps can begin before the
            # whole weight matrix lands.
            enc_s = singles.tile([128, KC, T_PER_CORE], f32, tag="enc")
            nc.sync.dma_start(out=enc_s, in_=encT)
            pred_s = singles.tile([128, KC, U], f32, tag="pred")
            nc.sync.dma_start(out=pred_s, in_=predT)
            bias_s = singles.tile([128, V], b_dt, tag="bias")
            nc.sync.dma_start(out=bias_s, in_=bias_d.ap().to_broadcast((128, V)))
            if dyn_iters:
                n_s = singles.tile([1, 1], mybir.dt.int32, tag="niters")
                nc.sync.dma_start(out=n_s, in_=n_d.ap())
            w_s = [singles.tile([128, V], w_dt, tag=f"w{k}", name=f"w{k}") for k in range(KC)]
            for k in range(KC):
                nc.sync.dma_start(out=w_s[k], in_=wT[:, k, :])

            def body():
                for m in range(M_CHUNKS):
                    hT = hpool.tile([128, KC, 128], h_dt, tag="hT")
                    for k in range(KC):
                        for j in range(T_PER_M):
                            t = m * T_PER_M + j
                            nc.scalar.activation(
                                out=hT[:, k, j * U : (j + 1) * U],
                                in_=pred_s[:, k, :],
                                func=mybir.ActivationFunctionType.Tanh,
                                bias=enc_s[:, k, t : t + 1],
                            )
                    psums = [
                        psum_pool.tile([128, 512], mybir.dt.float32, tag="ps", name="ps")
                        for _ in range(NB)
                    ]
                    # first two m-chunks sweep k-outer so each weight tile is
                    # consumed as it arrives; steady state goes n-outer so
                    # each psum bank completes (and drains) early.
                    if m < 2:
                        for k in range(KC):
                            for n in range(NB):
                                nc.tensor.matmul(
                                    psums[n],
                                    hT[:, k, :],
                                    w_s[k][:, n * 512 : (n + 1) * 512],
                                    start=(k == 0),
                                    stop=(k == KC - 1),
                                )
                    else:
                        for n in range(NB):
                            for k in range(KC):
                                nc.tensor.matmul(
                                    psums[n],
                                    hT[:, k, :],
                                    w_s[k][:, n * 512 : (n + 1) * 512],
                                    start=(k == 0),
                                    stop=(k == KC - 1),
                                )
                    if m < M_CHUNKS - 2:
                        obuf = opool.tile([128, V], f32, tag="obuf")
                        for n in range(NB):
                            nc.vector.tensor_add(
                                obuf[:, n * 512 : (n + 1) * 512],
                                psums[n],
                                bias_s[:, n * 512 : (n + 1) * 512],
                            )
                        nc.sync.dma_start(
                            out=out[m * 128 : (m + 1) * 128, :], in_=obuf
                        )
                    else:
                        # tail: drain+store per bank so the final DMA is 512
                        # cols, not 4096 — shortens the post-matmul tail.
                        for n in range(NB):
                            ob = lastp.tile([128, 512], f32, tag=f"lb{m % 2}_{n}", name="lb")
                            nc.vector.tensor_add(
                                ob, psums[n], bias_s[:, n * 512 : (n + 1) * 512]
                            )
                            nc.sync.dma_start(
                                out=out[m * 128 : (m + 1) * 128, n * 512 : (n + 1) * 512],
                                in_=ob,
                            )

            if dyn_iters:
                n_iters = nc.values_load(
                    n_s[0:1, 0:1], min_val=1, max_val=1 << 20,
                    skip_runtime_bounds_check=True,
                )
                with tc.For_i(0, n_iters) as _:
                    body()
                nc.sync.dma_start(out=probe_d.ap(), in_=bias_s[:, :512])
            else:
                body()

    nc.compile()
    return nc


_NC_CACHE = {}


def _get_nc(mode: str):
    if mode not in _NC_CACHE:
        _NC_CACHE[mode] = build_nc(mode)
    return _NC_CACHE[mode]


def make_in_maps(enc_out, pred_out, W_out, b_out, mode=None):
    mode = mode or MM_MODE
    wT = np.ascontiguousarray(W_out.T)  # [D, V]
    bias2d = np.ascontiguousarray(b_out.reshape(1, V))
    if mode == "bf16":
        import ml_dtypes

        wT = wT.astype(ml_dtypes.bfloat16)
        bias2d = bias2d.astype(ml_dtypes.bfloat16)
    in_maps = []
    for i in range(N_CORES):
        b_idx = i // (N_CORES // B)
        t0 = (i % (N_CORES // B)) * T_PER_CORE
        in_maps.append(
            {
                "encT": np.ascontiguousarray(enc_out[b_idx, t0 : t0 + T_PER_CORE].T),
                "predT": np.ascontiguousarray(pred_out[b_idx].T),
                "wT": wT,
                "bias": bias2d,
            }
        )
    return in_maps


def kernel(enc_out, pred_out, W_out, b_out):
    os.environ["BASS_NEVER_TRACE"] = "1"
    enc_out = np.asarray(enc_out, dtype=np.float32)
    pred_out = np.asarray(pred_out, dtype=np.float32)
    W_out = np.asarray(W_out, dtype=np.float32)
    b_out = np.asarray(b_out, dtype=np.float32)

    mode = MM_MODE
    nc = _get_nc(mode)
    in_maps = make_in_maps(enc_out, pred_out, W_out, b_out, mode)

    res = run_bass_kernel_spmd(nc, in_maps, core_ids=list(range(N_CORES)))

    out = np.empty((B, T, U, V), dtype=np.float32)
    for i in range(N_CORES):
        b_idx = i // (N_CORES // B)
        t0 = (i % (N_CORES // B)) * T_PER_CORE
        out[b_idx, t0 : t0 + T_PER_CORE] = res.results[i]["out"].reshape(
            T_PER_CORE, U, V
        )
    return out


# revision 18
# speedup vs baseline: 1.1344x; 1.1344x over previous
"""RNN-T Joint network kernel for Trainium2 (Bass/Tile), 8-core SPMD.

Problem: out[b,t,u,v] = tanh(enc[b,t,:] + pred[b,u,:]) @ W[v,:] + bias[v]
  B=4, T=256, U=64, D=640, V=4096  (fp32)

Sharding: data-parallel over (B,T). Core i handles b = i//2, t in
[(i%2)*128, (i%2)*128+128). Each core computes an [128*64, 4096] slice of
the output; no collectives needed.

Device kernel (per core):
  - host pre-transposes operands so the contraction dim D sits on SBUF
    partitions: encT [D,128], predT [D,64], wT [D,V].
  - hT[d, (t,u)] = tanh(predT[d,u] + encT[d,t]) via scalar-engine
    activation with per-partition bias (one instr per (d-chunk, t)).
  - PE matmul: psum[m128, n512] += hT[k][:,m].T @ wT[k][:,n] over 5
    k-chunks of 128; 8 psum banks cover V=4096 per 128-row chunk.
  - drain: vector add bias (broadcast tile) PSUM->SBUF, DMA out.
"""

import os
import sys

import numpy as np

if "/root/.axon_site/_ro/trn_rl_repo" not in sys.path:
    sys.path.append("/root/.axon_site/_ro/trn_rl_repo")

import concourse.bass as bass  # noqa: E402
import concourse.mybir as mybir  # noqa: E402
import concourse.tile as tile  # noqa: E402
from concourse import bacc  # noqa: E402
from concourse.bass_utils import run_bass_kernel_spmd  # noqa: E402

B, T, U, D, V = 4, 256, 64, 640, 4096
N_CORES = 8
T_PER_CORE = T // (N_CORES // B)  # 128
ROWS = T_PER_CORE * U  # 8192 rows per core
KC = D // 128  # 5 k-chunks
NB = V // 512  # 8 psum banks per row-chunk
M_CHUNKS = ROWS // 128  # 64  (each = 2 t values x 64 u)
T_PER_M = 128 // U  # 2

# matmul dtype mode: "f32" (exact, 4 cyc/row), "f32r" (1 cyc/row @N>=256),
# "bf16" (1 cyc/row, operands rounded to bf16, half the weight DMA)
MM_MODE = os.environ.get("JOINT_MM_MODE", "bf16")


def build_nc(mode: str, dyn_iters: bool = False):
    """dyn_iters=True adds a runtime `niters` input and wraps the whole
    per-core compute in a device-side For_i — used only for wall-clock
    timing (per-iter time = delta between two niters values)."""
    nc = bacc.Bacc("TRN2", target_bir_lowering=False, debug=False)

    f32 = mybir.dt.float32
    bf16 = mybir.dt.bfloat16
    f32r = mybir.dt.float32r
    w_dt = {"bf16": bf16, "f32r": f32r}.get(mode, f32)
    h_dt = w_dt

    b_dt = bf16 if mode == "bf16" else f32
    # output is written fp16 (halves the HBM write traffic, which binds the
    # 8-core aggregate) and upconverted to f32 on the host.
    o_dt = mybir.dt.float16 if mode == "bf16" else f32
    encT_d = nc.dram_tensor("encT", [D, T_PER_CORE], f32, kind="ExternalInput")
    predT_d = nc.dram_tensor("predT", [D, U], f32, kind="ExternalInput")
    wT_d = nc.dram_tensor("wT", [D, V], w_dt, kind="ExternalInput")
    bias_d = nc.dram_tensor("bias", [1, V], b_dt, kind="ExternalInput")
    if dyn_iters:
        n_d = nc.dram_tensor("niters", [1, 1], mybir.dt.int32, kind="ExternalInput")
        # timing build: full-size DRAM writes still happen (same DMA traffic),
        # but the 128MB buffer stays device-internal; only a small probe ships
        # back so wall-clock isn't drowned by tunnel transfer.
        out_d = nc.dram_tensor("out", [ROWS, V], o_dt, kind="Internal")
        probe_d = nc.dram_tensor("probe", [128, 512], b_dt, kind="ExternalOutput")
    else:
        out_d = nc.dram_tensor("out", [ROWS, V], o_dt, kind="ExternalOutput")

    encT = encT_d.ap().rearrange("(k p) t -> p k t", p=128)
    predT = predT_d.ap().rearrange("(k p) u -> p k u", p=128)
    wT = wT_d.ap().rearrange("(k p) v -> p k v", p=128)
    out = out_d.ap()

    with tile.TileContext(nc) as tc:
        with (
            tc.tile_pool(name="singles", bufs=1) as singles,
            tc.tile_pool(name="hpool", bufs=3) as hpool,
            tc.tile_pool(name="opool", bufs=3) as opool,
            tc.tile_pool(name="lastp", bufs=1) as lastp,
            tc.tile_pool(name="psum", bufs=8, space="PSUM") as psum_pool,
        ):
            # small operands first so the activation pipe starts immediately;
            # weights follow per-k so early k-sweeps can begin before the
            # whole weight matrix lands.
            enc_s = singles.tile([128, KC, T_PER_CORE], f32, tag="enc")
            nc.sync.dma_start(out=enc_s, in_=encT)
            pred_s = singles.tile([128, KC, U], f32, tag="pred")
            nc.sync.dma_start(out=pred_s, in_=predT)
            bias_s = singles.tile([128, V], b_dt, tag="bias")
            nc.sync.dma_start(out=bias_s, in_=bias_d.ap().to_broadcast((128, V)))
            if dyn_iters:
                n_s = singles.tile([1, 1], mybir.dt.int32, tag="niters")
                nc.sync.dma_start(out=n_s, in_=n_d.ap())
            w_s = [singles.tile([128, V], w_dt, tag=f"w{k}", name=f"w{k}") for k in range(KC)]
            for k in range(KC):
                nc.sync.dma_start(out=w_s[k], in_=wT[:, k, :])

            def body():
                for m in range(M_CHUNKS):
                    hT = hpool.tile([128, KC, 128], h_dt, tag="hT")
                    for k in range(KC):
                        for j in range(T_PER_M):
                            t = m * T_PER_M + j
                            nc.scalar.activation(
                                out=hT[:, k, j * U : (j + 1) * U],
                                in_=pred_s[:, k, :],
                                func=mybir.ActivationFunctionType.Tanh,
                                bias=enc_s[:, k, t : t + 1],
                            )
                    psums = [
                        psum_pool.tile([128, 512], mybir.dt.float32, tag="ps", name="ps")
                        for _ in range(NB)
                    ]
                    # first two m-chunks sweep k-outer so each weight tile is
                    # consumed as it arrives; steady state goes n-outer so
                    # each psum bank completes (and drains) early.
                    if m < 2:
                        for k in range(KC):
                            for n in range(NB):
                                nc.tensor.matmul(
                                    psums[n],
                                    hT[:, k, :],
                                    w_s[k][:, n * 512 : (n + 1) * 512],
                                    start=(k == 0),
                                    stop=(k == KC - 1),
                                )
                    else:
                        for n in range(NB):
                            for k in range(KC):
                                nc.tensor.matmul(
                                    psums[n],
                                    hT[:, k, :],
                                    w_s[k][:, n * 512 : (n + 1) * 512],
                                    start=(k == 0),
                                    stop=(k == KC - 1),
                                )
                    if m < M_CHUNKS - 2:
                        obuf = opool.tile([128, V], o_dt, tag="obuf")
                        for n in range(NB):
                            nc.vector.tensor_add(
                                obuf[:, n * 512 : (n + 1) * 512],
                                psums[n],
                                bias_s[:, n * 512 : (n + 1) * 512],
                            )
                        nc.sync.dma_start(
                            out=out[m * 128 : (m + 1) * 128, :], in_=obuf
                        )
                    else:
                        # tail: drain+store per bank so the final DMA is 512
                        # cols, not 4096 — shortens the post-matmul tail.
                        for n in range(NB):
                            ob = lastp.tile([128, 512], o_dt, tag=f"lb{m % 2}_{n}", name="lb")
                            nc.vector.tensor_add(
                                ob, psums[n], bias_s[:, n * 512 : (n + 1) * 512]
                            )
                            nc.sync.dma_start(
                                out=out[m * 128 : (m + 1) * 128, n * 512 : (n + 1) * 512],
                                in_=ob,
                            )

            if dyn_iters:
                n_iters = nc.values_load(
                    n_s[0:1, 0:1], min_val=1, max_val=1 << 20,
                    skip_runtime_bounds_check=True,
                )
                with tc.For_i(0, n_iters) as _:
                    body()
                nc.sync.dma_start(out=probe_d.ap(), in_=bias_s[:, :512])
            else:
                body()

    nc.compile()
    return nc


_NC_CACHE = {}


def _get_nc(mode: str):
    if mode not in _NC_CACHE:
        _NC_CACHE[mode] = build_nc(mode)
    return _NC_CACHE[mode]


def make_in_maps(enc_out, pred_out, W_out, b_out, mode=None):
    mode = mode or MM_MODE
    wT = np.ascontiguousarray(W_out.T)  # [D, V]
    bias2d = np.ascontiguousarray(b_out.reshape(1, V))
    if mode == "bf16":
        import ml_dtypes

        wT = wT.astype(ml_dtypes.bfloat16)
        bias2d = bias2d.astype(ml_dtypes.bfloat16)
    in_maps = []
    for i in range(N_CORES):
        b_idx = i // (N_CORES // B)
        t0 = (i % (N_CORES // B)) * T_PER_CORE
        in_maps.append(
            {
                "encT": np.ascontiguousarray(enc_out[b_idx, t0 : t0 + T_PER_CORE].T),
                "predT": np.ascontiguousarray(pred_out[b_idx].T),
                "wT": wT,
                "bias": bias2d,
            }
        )
    return in_maps


def kernel(enc_out, pred_out, W_out, b_out):
    os.environ["BASS_NEVER_TRACE"] = "1"
    enc_out = np.asarray(enc_out, dtype=np.float32)
    pred_out = np.asarray(pred_out, dtype=np.float32)
    W_out = np.asarray(W_out, dtype=np.float32)
    b_out = np.asarray(b_out, dtype=np.float32)

    mode = MM_MODE
    nc = _get_nc(mode)
    in_maps = make_in_maps(enc_out, pred_out, W_out, b_out, mode)

    res = run_bass_kernel_spmd(nc, in_maps, core_ids=list(range(N_CORES)))

    out = np.empty((B, T, U, V), dtype=np.float32)
    for i in range(N_CORES):
        b_idx = i // (N_CORES // B)
        t0 = (i % (N_CORES // B)) * T_PER_CORE
        # device writes fp16 in bf16 mode; assignment upcasts to f32
        out[b_idx, t0 : t0 + T_PER_CORE] = res.results[i]["out"].reshape(
            T_PER_CORE, U, V
        )
    return out
